# revision 4
# baseline (speedup 1.0000x reference)
"""BinaryLinear Trainium2 kernel.

Computes: out = binarize(x) @ binarize(weight - threshold).T * 2^round(clip(shift, -8, 0))

where binarize(v) = +1 if v >= 0 else -1, over x [B,S,IN], weight [OUT,IN].

Strategy (8 NeuronCores, tensor-parallel over OUT):
  - each core gets the full x and a 2048-row slice of weight/threshold
  - binarize to +/-0.5 (exact in bf16/fp8) with one fused DVE op; the
    missing x4 is folded into the final output scale
  - hardware DMA-transpose (bf16, xbar) produces the [contraction-
    partition] layout both matmul operands need, off the PE critical path
  - fp8 matmuls (values +/-0.5 exact in e4m3) accumulate into fp32 PSUM;
    optional DoubleRow perf mode contracts 256 rows/matmul for 2x PE rate
  - PSUM -> SBUF copy applies the power-of-two scale; result is bit-exact
"""

import sys

if "/opt/trn_rl_repo" not in sys.path:
    sys.path.insert(0, "/opt/trn_rl_repo")

import numpy as np

B, S, IN, OUT = 4, 2048, 4096, 16384
N_CORES = 8
O_SHARD = OUT // N_CORES  # 2048
P = 128  # partitions
N_CH = 512  # psum free-dim chunk (one bank of fp32)

USE_DOUBLE_ROW = True

# dev knobs (test.py only; harness uses defaults)
_TRACE = False
_LAST_RESULTS = None


def build_program(s_rows=B * S, o_shard=O_SHARD, kdim=IN, scale=1.0,
                  use_dr=USE_DOUBLE_ROW):
    """Trace the single-core SPMD program. Inputs: x [s_rows,kdim] f32,
    w [o_shard,kdim] f32, thr [o_shard,1] f32. Output: out [s_rows,o_shard] f32."""
    import concourse.bass as bass
    import concourse.mybir as mybir
    import concourse.tile as tile
    from concourse import bacc
    from concourse.alu_op_type import AluOpType

    f32 = mybir.dt.float32
    bf16 = mybir.dt.bfloat16
    fp8 = mybir.dt.float8e4

    n_sb = s_rows // P
    n_kt = kdim // P
    n_ob = o_shard // P
    n_oc = o_shard // N_CH

    nc = bacc.Bacc(None, target_bir_lowering=False, debug=False)

    x_d = nc.dram_tensor("x", [s_rows, kdim], f32, kind="ExternalInput")
    w_d = nc.dram_tensor("w", [o_shard, kdim], f32, kind="ExternalInput")
    t_d = nc.dram_tensor("thr", [o_shard, 1], f32, kind="ExternalInput")
    o_d = nc.dram_tensor("out", [s_rows, o_shard], f32, kind="ExternalOutput")

    with tile.TileContext(nc) as tc:
        with (
            tc.tile_pool(name="raw", bufs=3) as raw_pool,
            tc.tile_pool(name="b16", bufs=2) as b16_pool,
            tc.tile_pool(name="t16", bufs=2) as t16_pool,
            tc.tile_pool(name="w8", bufs=1) as w8_pool,
            tc.tile_pool(name="x8", bufs=3) as x8_pool,
            tc.tile_pool(name="outp", bufs=2) as out_pool,
            tc.tile_pool(name="thrp", bufs=2) as thr_pool,
            tc.tile_pool(name="ps", bufs=2, space="PSUM") as ps_pool,
        ):
            # --- weight prep: binarize + transpose into fp8 slab [p, kt, o] ---
            wslab = w8_pool.tile([P, n_kt, o_shard], fp8)
            for ob in range(n_ob):
                w_raw = raw_pool.tile([P, kdim], f32, name="w_raw", tag="raw")
                nc.sync.dma_start(w_raw[:], w_d[ob * P:(ob + 1) * P, :])
                thr_t = thr_pool.tile([P, 1], f32, name="thr_t", tag="thr")
                nc.sync.dma_start(thr_t[:], t_d[ob * P:(ob + 1) * P, :])
                wb16 = b16_pool.tile([P, kdim], bf16, name="wb16", tag="b16")
                # (w >= thr) - 0.5  ->  +/-0.5, exact
                nc.vector.tensor_scalar(
                    wb16[:], w_raw[:], thr_t[:], 0.5,
                    AluOpType.is_ge, AluOpType.subtract)
                wt16 = t16_pool.tile([P, n_kt, P], bf16, name="wt16", tag="t16")
                nc.sync.dma_start(wt16[:], wb16[:], transpose=True)
                nc.vector.tensor_copy(wslab[:, :, ob * P:(ob + 1) * P], wt16[:])

            # --- main loop over s-blocks ---
            for sb in range(n_sb):
                x_raw = raw_pool.tile([P, kdim], f32, name="x_raw", tag="raw")
                nc.sync.dma_start(x_raw[:], x_d[sb * P:(sb + 1) * P, :])
                xb16 = b16_pool.tile([P, kdim], bf16, name="xb16", tag="b16")
                nc.vector.tensor_scalar(
                    xb16[:], x_raw[:], 0.0, 0.5,
                    AluOpType.is_ge, AluOpType.subtract)
                xt16 = t16_pool.tile([P, n_kt, P], bf16, name="xt16", tag="t16")
                nc.sync.dma_start(xt16[:], xb16[:], transpose=True)
                x8 = x8_pool.tile([P, n_kt, P], fp8, name="x8", tag="x8")
                nc.vector.tensor_copy(x8[:], xt16[:])

                out_sb = out_pool.tile([P, o_shard], f32, name="out_sb", tag="out")
                pss = [
                    ps_pool.tile([P, N_CH], f32, name=f"ps{oc}", tag=f"ps{oc}")
                    for oc in range(n_oc)
                ]
                if use_dr:
                    assert n_kt % 2 == 0
                    for g in range(n_kt // 2):
                        for oc in range(n_oc):
                            nc.tensor.matmul(
                                pss[oc][:],
                                x8[:, 2 * g:2 * g + 2, :],
                                wslab[:, 2 * g:2 * g + 2, oc * N_CH:(oc + 1) * N_CH],
                                start=(g == 0), stop=(g == n_kt // 2 - 1),
                                perf_mode=mybir.MatmulPerfMode.DoubleRow)
                else:
                    for kt in range(n_kt):
                        for oc in range(n_oc):
                            nc.tensor.matmul(
                                pss[oc][:],
                                x8[:, kt, :],
                                wslab[:, kt, oc * N_CH:(oc + 1) * N_CH],
                                start=(kt == 0), stop=(kt == n_kt - 1))
                for oc in range(n_oc):
                    nc.scalar.activation(
                        out_sb[:, oc * N_CH:(oc + 1) * N_CH], pss[oc][:],
                        mybir.ActivationFunctionType.Copy,
                        bias=0.0, scale=float(scale))
                nc.sync.dma_start(o_d[sb * P:(sb + 1) * P, :], out_sb[:])

    nc.compile()
    return nc


def _host_scale(shift_param):
    # 4x undoes the two 0.5 factors from binarizing to +/-0.5;
    # np.round is round-half-to-even, matching jnp.round.
    s = np.clip(np.float64(np.float32(shift_param)), -8.0, 0.0)
    return 4.0 * float(np.exp2(np.round(s)))


def kernel(x, weight, threshold, shift_param):
    from concourse.bass_utils import run_bass_kernel_spmd

    scale = _host_scale(shift_param)
    nc = build_program(scale=scale)

    xf = np.ascontiguousarray(x.astype(np.float32).reshape(B * S, IN))
    in_maps = []
    for c in range(N_CORES):
        sl = slice(c * O_SHARD, (c + 1) * O_SHARD)
        in_maps.append({
            "x": xf,
            "w": np.ascontiguousarray(weight[sl].astype(np.float32)),
            "thr": np.ascontiguousarray(
                threshold[sl].astype(np.float32).reshape(O_SHARD, 1)),
        })

    res = run_bass_kernel_spmd(nc, in_maps, list(range(N_CORES)), trace=_TRACE)
    global _LAST_RESULTS
    _LAST_RESULTS = res
    shards = [res.results[c]["out"] for c in range(N_CORES)]
    full = np.concatenate(shards, axis=1).reshape(B, S, OUT)
    return np.ascontiguousarray(full.astype(np.float32))
